# revision 53
# baseline (speedup 1.0000x reference)
"""AttentionBlock (GroupNorm + single-head full attention + residual) on 8
Trainium2 NeuronCores.

Sharding: data-parallel over batch (4) x sequence-parallel over query
tokens (2 halves of h*w=4096). Each core gets its batch slice with the
token axis ROTATED by the host so that its 2048 queries are always
columns 0:NQ (attention is permutation-invariant over keys, GroupNorm
over positions). No collectives; the host scatters inputs and gathers
outputs.

Per-core pipeline (channels on partitions; attention entirely in fp8e4
DoubleRow = 2 contraction rows/cell on the PE):
 - GroupNorm folded into the weights: W' = W*a per input channel
   (a = gamma*rstd), so Q/K/V2 consume RAW x. Stats via DVE
   bn_stats/bn_aggr; rstd via a DVE-only Newton rsqrt (bit-trick seed),
   so ScalarE runs NOTHING but exp (single act-table set, never
   reloaded).
 - Q/K projected straight into fp8 DoubleRow layout [128, 2, n]
   (contraction row c = i*128+p). K carries NO bias: its score term is
   constant along the softmax axis and cancels exactly. Q's bias is
   added in PSUM by a rank-1 matmul (qb^T x ones-row), so every
   projection write is a plain DVE TensorCopy (DVE fp8 TensorScalar
   ops lock up the HW; TensorCopy is safe).
 - S^T per k-tile is ONE DoubleRow matmul; k-tile PAIRS share a 2-bank
   PSUM tile so exp runs as a single [128, 1024] ScalarE op into the
   fp8 pT pair. P*V also DoubleRow, s-outer, accumulating
   [q, 256 c + l] with the softmax denominator in an appended ones
   column of V2^T; 1/l scaling on DVE, PE transpose back to [c, q].
 - out_w folded into V (W2 = out_w @ Wv on device); weights arrive
   pre-transposed from the host.

Cross-iteration software pipeline (loop_reps): weights/consts load once
in setup; x(N+1) DMAs issue at body-N start (own SWDGE queue, never
behind the y-output queue); bn-stats(N+1) runs mid-attention of N;
fold after PV1(N), proj after PV2(N); S(N+1, qc0) is emitted before
PV3(N). The ScalarE exp stream -- the bottleneck at ~97% busy -- runs
back-to-back across iterations; per-qc epilogue transposes are
deferred past all four po streams so PE pays the o_sb DVE-latency
wait once per qc. Steady-state ~67.8 us/iter in TimelineSim
(baseline 146.4 us), ~1038 ns per [128,1024] exp = 853 processing +
185 SBUF-access penalty; residual ACT gaps ~1.0 us/iter.

Toolchain notes: walrus accepts at most one sync-wait per instruction
(SplitWaitTileContext splits the rest onto nops); f32r matmul inputs
must be PRODUCED as f32r (casting gpsimd DMAs / engine copies) -- a
bitcast of plain f32 fails the BIR verifier; DoubleRow needs fp8 on
both operands, 3D APs [Ki, 2, dim] with 16B-aligned middle stride;
bitwise and arith ALU ops cannot share one dual-op tensor_scalar.
"""

import numpy as np

B, C, HW = 4, 256, 4096
NQ = HW // 2
G = 8
CPG = C // G  # channels per group
EPS = 1e-5
N_CORES = 8
USE_FP8_PV = True
USE_FP8_QK = True
EXP_BIAS = -3.0

_CACHE = {}


def _build_nc(loop_reps=1, debug=False):
    import bass_rust
    import concourse.bass as bass
    import concourse.mybir as mybir
    import concourse.tile as tile
    from concourse.masks import make_identity
    from concourse.vector_clock import ScopedClock

    F32 = mybir.dt.float32
    FR = mybir.dt.float32r
    F8 = mybir.dt.float8e4
    AF = mybir.ActivationFunctionType
    ALU = mybir.AluOpType

    MAXW = 1

    class SplitWaitTileContext(tile.TileContext):
        """Workaround: this toolchain's walrus accepts at most one sync-wait
        per instruction; split excess waits onto same-engine InstNoOps."""

        def _split_excess_waits(self, inst):
            si = inst.sync_info
            if si is None:
                return []
            waits = list(si.on_wait)
            if len(waits) <= MAXW:
                return []
            extra, keep = waits[:-MAXW], waits[-MAXW:]
            nops = [
                mybir.InstNoOp(
                    name=f"I-{self.nc.next_id()}",
                    sync_info=mybir.SyncInfo(on_wait=[w], on_update=[]),
                    bass_nofuse=True,
                    engine=inst.engine,
                )
                for w in extra
            ]
            inst.sync_info = mybir.SyncInfo(on_wait=keep, on_update=list(si.on_update))
            return nops

        def _commit_and_lower(self, inst, original_block, old_bb_map, bb_to_exit_bb):
            for nop in self._split_excess_waits(inst):
                self._commit_instruction(nop, lazy_reg_writes=False)
            return super()._commit_and_lower(
                inst, original_block, old_bb_map, bb_to_exit_bb
            )

        def _drain_and_barrier(self, tick_clock, wait_clock):
            drain_inst = self.nc.sync.drain()
            wait_clock.add_sem_waits(
                drain_inst.ins, ScopedClock({None: tick_clock.global_clock})
            )
            si = drain_inst.ins.sync_info
            waits = list(si.on_wait) if si is not None else []
            if len(waits) > MAXW:
                updates = list(si.on_update) if si is not None else []
                drain_inst.ins.sync_info = bass_rust.SyncInfo(
                    on_wait=waits[:MAXW], on_update=[]
                )
                rest = waits[MAXW:]
                for i, w in enumerate(rest):
                    extra = self.nc.sync.drain()
                    extra.ins.sync_info = bass_rust.SyncInfo(
                        on_wait=[w], on_update=updates if i == len(rest) - 1 else []
                    )
            self.nc.all_engine_barrier()
            assert self.sems is not None
            popped = self.nc._tile_sem_poison_stack.pop()
            assert popped is self._sem_poison
            self.nc.clear_and_free_semaphores(list(self.sems.allocated().values()))
            self.nc.all_engine_barrier()

    nc = bass.Bass()
    xb = nc.dram_tensor("xb", [C, HW], F32, kind="ExternalInput")
    qkv_w = nc.dram_tensor("qkv_w", [3 * C, C], F32, kind="ExternalInput")
    qkv_b = nc.dram_tensor("qkv_b", [3 * C], F32, kind="ExternalInput")
    out_w = nc.dram_tensor("out_w", [C, C], F32, kind="ExternalInput")
    out_b = nc.dram_tensor("out_b", [C], F32, kind="ExternalInput")
    gn_gamma = nc.dram_tensor("gn_gamma", [C], F32, kind="ExternalInput")
    gn_beta = nc.dram_tensor("gn_beta", [C], F32, kind="ExternalInput")
    gind_in = nc.dram_tensor("gind_in", [128, 16], F32, kind="ExternalInput")
    hind_in = nc.dram_tensor("hind_in", [8, 128 * 2], F32, kind="ExternalInput")
    ones_in = nc.dram_tensor(
        "ones_in", [128, 256], F8 if USE_FP8_PV else F32, kind="ExternalInput"
    )
    ident_in = nc.dram_tensor("ident_in", [128, 128], F32, kind="ExternalInput")
    onesr_in = nc.dram_tensor("onesr_in", [1, 512], F32, kind="ExternalInput")
    wqkT_in = nc.dram_tensor("wqkT_in", [C, 512], F32, kind="ExternalInput")
    owT_in = nc.dram_tensor("owT_in", [C, C], F32, kind="ExternalInput")
    y = nc.dram_tensor("y", [C, NQ], F32, kind="ExternalOutput")
    if debug:
        d_xn = nc.dram_tensor("d_xn", [C, HW], F32, kind="ExternalOutput")
        d_q = nc.dram_tensor("d_q", [C, NQ], F32, kind="ExternalOutput")
        d_k = nc.dram_tensor("d_k", [C, HW], F32, kind="ExternalOutput")
        d_v2t = nc.dram_tensor("d_v2t", [HW, 272], F32, kind="ExternalOutput")
        d_po = nc.dram_tensor("d_po", [128, 272], F32, kind="ExternalOutput")
        d_ab = nc.dram_tensor("d_ab", [C, 2], F32, kind="ExternalOutput")

    with SplitWaitTileContext(nc) as tc:
        import contextlib

        ctx = contextlib.ExitStack()
        with ctx:
            singles = ctx.enter_context(tc.tile_pool(name="singles", bufs=1))
            xpool = ctx.enter_context(tc.tile_pool(name="xpool", bufs=2))
            qpool = ctx.enter_context(tc.tile_pool(name="qpool", bufs=2))
            kpool = ctx.enter_context(tc.tile_pool(name="kpool", bufs=2))
            vpool = ctx.enter_context(tc.tile_pool(name="vpool", bufs=2))
            ypool = ctx.enter_context(tc.tile_pool(name="ypool", bufs=2))
            wpool = ctx.enter_context(tc.tile_pool(name="wpool", bufs=1))
            wnat = ctx.enter_context(tc.tile_pool(name="wnat", bufs=1))
            ppool = ctx.enter_context(tc.tile_pool(name="ppool", bufs=2))
            opool = ctx.enter_context(tc.tile_pool(name="opool", bufs=6))
            small = ctx.enter_context(tc.tile_pool(name="small", bufs=8))
            stat = ctx.enter_context(tc.tile_pool(name="stat", bufs=2))
            pspair = ctx.enter_context(tc.tile_pool(name="pspair", bufs=2, space="PSUM"))
            psov = ctx.enter_context(tc.tile_pool(name="psov", bufs=2, space="PSUM"))
            pstp = ctx.enter_context(tc.tile_pool(name="pstp", bufs=2, space="PSUM"))
            psmm = pspair  # head smalls share the pair-tile rotation

            wsc = ctx.enter_context(tc.tile_pool(name="wsc", bufs=2))

            def setup():
                """Iteration-invariant work: constants, raw weights, the
                W2 = out_w @ Wv fold and ob_eff. Loaded/computed ONCE per
                build; the rep loop only touches x-dependent state."""
                ident = singles.tile([128, 128], FR, tag="ident")
                eps_sb = singles.tile([8, 1], F32, tag="eps")
                nc.vector.memset(eps_sb, EPS)
                ebias_sb = singles.tile([128, 1], F32, tag="ebias")
                nc.vector.memset(ebias_sb, EXP_BIAS)
                gam_sb = singles.tile([128, 2], F32, tag="gam")
                bet_sb = singles.tile([128, 2], F32, tag="bet")
                qb_sb = singles.tile([128, 6], F32, tag="qb")
                ob_sb = singles.tile([128, 2], F32, tag="ob")
                gi_sb = singles.tile([128, 16], F32, tag="gi")
                hi_sb = singles.tile([8, 128 * 2], F32, tag="hi")
                nc.sync.dma_start(out=qb_sb, in_=qkv_b.rearrange("(m p) -> p m", p=128))
                nc.sync.dma_start(out=gam_sb, in_=gn_gamma.rearrange("(t p) -> p t", p=128))
                nc.sync.dma_start(out=bet_sb, in_=gn_beta.rearrange("(t p) -> p t", p=128))
                nc.sync.dma_start(out=ob_sb, in_=out_b.rearrange("(t p) -> p t", p=128))
                nc.sync.dma_start(out=gi_sb, in_=gind_in[:, :])
                nc.sync.dma_start(out=hi_sb, in_=hind_in[:, :])
                nc.gpsimd.dma_start(out=ident, in_=ident_in[:, :])
                onesr = singles.tile([1, 512], FR, tag="onesr")
                nc.gpsimd.dma_start(out=onesr, in_=onesr_in[:, :])

                wT = []  # (Wq|Wk)^T tiles [c_in 128, 512] f32r
                owT = []  # out_w^T tiles [c_in 128, 256] f32r
                for t in range(2):
                    wT.append(wpool.tile([128, 512], FR, tag=f"wT{t}", name=f"wTn{t}"))
                    owT.append(wpool.tile([128, 256], FR, tag=f"owT{t}", name=f"owT{t}"))
                for t in range(2):
                    nc.gpsimd.dma_start(out=wT[t], in_=wqkT_in[t * 128 : (t + 1) * 128, :])
                    nc.gpsimd.dma_start(out=owT[t], in_=owT_in[t * 128 : (t + 1) * 128, :])
                wv_fr = []
                for i in range(2):
                    wv = wpool.tile([128, C], FR, tag=f"wv{i}", name=f"wv{i}")
                    nc.gpsimd.dma_start(
                        out=wv, in_=qkv_w[512 + i * 128 : 512 + (i + 1) * 128, :]
                    )
                    wv_fr.append(wv)
                w2t = []
                for t in range(2):
                    ps = psmm.tile([128, 256], F32, tag="mm")
                    nc.tensor.matmul(
                        ps, wv_fr[0][:, t * 128 : (t + 1) * 128], owT[0],
                        start=True, stop=False,
                    )
                    nc.tensor.matmul(
                        ps, wv_fr[1][:, t * 128 : (t + 1) * 128], owT[1],
                        start=False, stop=True,
                    )
                    w2 = wpool.tile([128, 256], FR, tag=f"w2t{t}", name=f"w2t{t}")
                    nc.vector.tensor_copy(w2, ps)
                    w2t.append(w2)
                # ob_eff = out_b + out_w @ bv  (bv = qkv_b[512:768])
                ps_ob = psmm.tile([128, 2], F32, tag="mm")
                for m2 in range(2):
                    nc.tensor.matmul(
                        ps_ob[:, m2 : m2 + 1],
                        owT[0][:, m2 * 128 : (m2 + 1) * 128].bitcast(F32),
                        qb_sb[:, 4:5],
                        start=True, stop=False,
                    )
                    nc.tensor.matmul(
                        ps_ob[:, m2 : m2 + 1],
                        owT[1][:, m2 * 128 : (m2 + 1) * 128].bitcast(F32),
                        qb_sb[:, 5:6],
                        start=False, stop=True,
                    )
                ob_eff = singles.tile([128, 2], F32, tag="obeff")
                nc.vector.tensor_add(ob_eff, ps_ob, ob_sb)
                return dict(
                    ident=ident, eps_sb=eps_sb, ebias_sb=ebias_sb, onesr=onesr,
                    g_sb=[gam_sb[:, t : t + 1] for t in range(2)],
                    be_sb=[bet_sb[:, t : t + 1] for t in range(2)],
                    qb_sb=qb_sb, ob_eff=ob_eff,
                    gind=[gi_sb[:, 0:8], gi_sb[:, 8:16]],
                    hind=[hi_sb[:, 0:128], hi_sb[:, 128:256]],
                    wT=wT, w2t=w2t,
                )

            def load_x(C_):
                """x for one iteration, split across two DMA queues (Pool
                swdge + DVE queue) so the c-halves stream in parallel and
                never sit behind the y-output queue (sync)."""
                x_sb = []
                for t in range(2):
                    xt = xpool.tile([128, HW], FR, tag="xv", name=f"x{t}")
                    q = nc.gpsimd
                    for c4 in range(4):
                        q.dma_start(
                            out=xt[:, c4 * 1024 : (c4 + 1) * 1024],
                            in_=xb[t * 128 : (t + 1) * 128, c4 * 1024 : (c4 + 1) * 1024],
                        )
                    x_sb.append(xt)
                return x_sb

            def stats_part1(C_, x_sb):
                """Pure-DVE GroupNorm stats (bn_stats/bn_aggr -> per-channel
                mean, E[x^2]). Emitted early so it runs during the previous
                iteration's exp stream without stalling PE or ACT."""
                st2 = []
                for t in range(2):
                    bstat = stat.tile([128, 8, 6], F32, tag=f"bst{t}", name=f"bst{t}")
                    for cck in range(8):
                        nc.vector.bn_stats(
                            out=bstat[:, cck, :],
                            in_=x_sb[t][:, cck * 512 : (cck + 1) * 512].bitcast(F32),
                        )
                    mv = stat.tile([128, 2], F32, tag=f"mv{t}", name=f"mv{t}")
                    nc.vector.bn_aggr(out=mv, in_=bstat)
                    s2t = stat.tile([128, 2], F32, tag=f"st2{t}")
                    nc.vector.tensor_copy(s2t[:, 0:1], mv[:, 0:1])
                    nc.vector.scalar_tensor_tensor(
                        out=s2t[:, 1:2], in0=mv[:, 0:1], scalar=mv[:, 0:1],
                        in1=mv[:, 1:2], op0=ALU.mult, op1=ALU.add,
                    )
                    st2.append(s2t)
                return st2, x_sb

            def fold_part2(C_, p1):
                """Group reduction (tiny PE matmuls), DVE rsqrt, and the
                GN-into-weights fold. Emitted late (after S3) when the
                stats are long since ready, so the in-order PE queue never
                stalls on them."""
                st2, x_sb = p1
                psg = pstp.tile([8, 2], F32, tag="pst", name="psg")
                nc.tensor.matmul(psg, C_["gind"][0], st2[0], start=True, stop=False)
                nc.tensor.matmul(psg, C_["gind"][1], st2[1], start=False, stop=True)
                gstat = stat.tile([8, 2], F32, tag="gstat")  # [mean_g, E[x^2]_g]
                nc.vector.tensor_scalar_mul(gstat, psg, 1.0 / CPG)
                var_g = stat.tile([8, 1], F32, tag="varg")
                nc.vector.tensor_mul(var_g, gstat[:, 0:1], gstat[:, 0:1])
                nc.vector.tensor_sub(var_g, gstat[:, 1:2], var_g)
                # rstd = rsqrt(var+eps) fully on DVE (bit-trick seed + 3
                # Newton steps on an [8,1] tile) so ACT's in-order stream
                # carries ONLY exp/identity work and never stalls on stats.
                I32 = mybir.dt.int32
                vpe = stat.tile([8, 1], F32, tag="vpe")
                nc.vector.tensor_scalar_add(vpe, var_g, EPS)
                rm = stat.tile([8, 2], F32, tag="rm")  # [rstd_g, mean_g]
                ry = rm[:, 0:1]
                nc.vector.tensor_scalar(
                    out=ry.bitcast(I32), in0=vpe.bitcast(I32),
                    scalar1=1, scalar2=None,
                    op0=ALU.logical_shift_right,
                )
                nc.vector.tensor_scalar(
                    out=ry.bitcast(I32), in0=ry.bitcast(I32),
                    scalar1=-1, scalar2=0x5F3759DF,
                    op0=ALU.mult, op1=ALU.add,
                )
                nt_ = stat.tile([8, 1], F32, tag="nt")
                for _ in range(3):
                    nc.vector.tensor_mul(nt_, ry, ry)
                    nc.vector.tensor_mul(nt_, nt_, vpe)
                    nc.vector.tensor_scalar(
                        out=nt_, in0=nt_, scalar1=-0.5, scalar2=1.5,
                        op0=ALU.mult, op1=ALU.add,
                    )
                    nc.vector.tensor_mul(ry, ry, nt_)
                nc.vector.tensor_copy(rm[:, 1:2], gstat[:, 0:1])
                # broadcast to channels: [rstd_c, mean_c] = H_t.T @ rm
                ab = []
                for t in range(2):
                    psb = pstp.tile([128, 2], F32, tag="pst", name="psb")
                    nc.tensor.matmul(psb, C_["hind"][t], rm, start=True, stop=True)
                    abt = stat.tile([128, 2], F32, tag=f"ab{t}")  # [a_c, b_c]
                    nc.vector.tensor_mul(abt[:, 0:1], psb[:, 0:1], C_["g_sb"][t])
                    nc.vector.tensor_mul(abt[:, 1:2], psb[:, 1:2], abt[:, 0:1])
                    nc.vector.tensor_sub(abt[:, 1:2], C_["be_sb"][t], abt[:, 1:2])
                    ab.append(abt)

                # fold GN into weights: W' = W * a (per c_in); bias matmuls
                # (plain fp32, N=1-2) use the UNSCALED weights.
                wTs, w2ts = [], []
                for t in range(2):
                    wt2 = wsc.tile([128, 512], FR, tag=f"wTs{t}", name=f"wTs{t}")
                    nc.vector.tensor_scalar_mul(wt2, C_["wT"][t], ab[t][:, 0:1])
                    wTs.append(wt2)
                    w22 = wsc.tile([128, 256], FR, tag=f"w2ts{t}", name=f"w2ts{t}")
                    nc.vector.tensor_scalar_mul(w22, C_["w2t"][t], ab[t][:, 0:1])
                    w2ts.append(w22)
                ps_qb = pstp.tile([128, 2], F32, tag="pst", name="ps_qb")
                for m in range(2):  # only the Q bias survives softmax
                    nc.tensor.matmul(
                        ps_qb[:, m : m + 1],
                        C_["wT"][0][:, m * 128 : (m + 1) * 128].bitcast(F32),
                        ab[0][:, 1:2],
                        start=True, stop=False,
                    )
                    nc.tensor.matmul(
                        ps_qb[:, m : m + 1],
                        C_["wT"][1][:, m * 128 : (m + 1) * 128].bitcast(F32),
                        ab[1][:, 1:2],
                        start=False, stop=True,
                    )
                qb_eff = stat.tile([128, 2], FR, tag="qbeff")
                nc.vector.tensor_add(qb_eff, ps_qb, C_["qb_sb"][:, 0:2])
                # transposed Q bias [2, 128] f32r: lhsT of the rank-1
                # bias matmul folded into each Q projection psum, which
                # turns ALL projection writes into plain DVE copies (no
                # ACT Identity+bias work in the exp stream).
                qbT = []
                for m in range(2):
                    ps_qbT = pstp.tile([1, 128], FR, tag="pst", name="ps_qbT")
                    nc.tensor.transpose(ps_qbT, qb_eff[:, m : m + 1], C_["ident"])
                    qt = stat.tile([1, 128], FR, tag=f"qbT{m}")
                    nc.vector.tensor_copy(qt, ps_qbT)
                    qbT.append(qt)
                ps_ob2 = pstp.tile([128, 2], F32, tag="pst", name="ps_ob2")
                for m2 in range(2):
                    nc.tensor.matmul(
                        ps_ob2[:, m2 : m2 + 1],
                        C_["w2t"][0][:, m2 * 128 : (m2 + 1) * 128].bitcast(F32),
                        ab[0][:, 1:2],
                        start=True, stop=False,
                    )
                    nc.tensor.matmul(
                        ps_ob2[:, m2 : m2 + 1],
                        C_["w2t"][1][:, m2 * 128 : (m2 + 1) * 128].bitcast(F32),
                        ab[1][:, 1:2],
                        start=False, stop=True,
                    )
                ob_f = stat.tile([128, 2], F32, tag="obf")
                nc.vector.tensor_add(ob_f, ps_ob2, C_["ob_eff"])

                # residual prep from raw xq
                y_sb = []
                for t in range(2):
                    yt = ypool.tile([128, NQ], F32, tag="y", name=f"y{t}")
                    nc.vector.tensor_scalar_add(yt, x_sb[t][:, 0:NQ].bitcast(F32), ob_f[:, t : t + 1])
                    y_sb.append(yt)
                return dict(x=x_sb, wTs=wTs, w2ts=w2ts, qbT=qbT, y_sb=y_sb)

            NPAD = 16
            NV2 = 256 + NPAD
            NQC = NQ // 512

            def emit_proj(C_, st):
                # Q/K as fp8e4 in DoubleRow layout [128, 2, n] (contraction
                # row c = i*128 + p for the S^T matmuls). K carries NO bias:
                # S = (q+bq).k + (q+bq).bk and the bk term is constant over
                # the softmax axis, so it cancels exactly -- K psum pairs are
                # plain DVE copies (DVE fp8 TensorCopy is HW-safe, fp8
                # TensorScalar is not); Q keeps its bias via ACT Identity.
                q8 = qpool.tile([128, 2, NQ], F8, tag="q", name="q8")
                k8 = kpool.tile([128, 2, HW], F8, tag="k", name="k8")
                x_sb = st["x"]
                # K/Q chunk order: K j0 (both c-halves) then Q j0 first, so
                # S(qc0, j0..1) and the exp stream start after 4 chunk
                # writes. Proj psums use the psov 1-bank rotation -- the
                # "mm" pair rotation is reserved for S psums, so next-iter
                # S tiles never queue behind the proj drain.
                order = []
                for j in range(8):
                    order += [(2, j), (3, j)]
                    if j < 4:
                        order += [(0, j), (1, j)]
                for m, j in order:
                    ps = psov.tile([128, 512], F32, tag="o", name="psproj")
                    is_q = m < 2
                    for t in range(2):
                        nc.tensor.matmul(
                            ps,
                            st["wTs"][t][:, m * 128 : (m + 1) * 128],
                            x_sb[t][:, j * 512 : (j + 1) * 512],
                            start=(t == 0),
                            stop=(t == 1 and not is_q),
                        )
                    if is_q:
                        # rank-1 bias: qbT[m] (x) ones -- bias lands in psum
                        nc.tensor.matmul(
                            ps, st["qbT"][m], C_["onesr"],
                            start=False, stop=True,
                        )
                        dst = q8[:, m, j * 512 : (j + 1) * 512]
                    else:
                        dst = k8[:, m - 2, j * 512 : (j + 1) * 512]
                    nc.vector.tensor_copy(dst, ps)
                return q8, k8

            def emit_v2(C_, st, v2t):
                # psov pool (1-bank tiles), NOT the "mm" rotation: V2 psums
                # between S0's and S1's would stall the exp stream. v2t is a
                # persistent ping-pong buffer pair whose ones column (256:)
                # was initialized once in setup -- only [:, :, 0:256] is
                # rewritten here, so no per-iteration ones DMA exists to
                # block the Pool queue.
                x_sb = st["x"]
                for g in range(16):  # 2 n-tiles per psum tile
                    ps = psov.tile([128, 2, 256], F32, tag="o", name="psv2")
                    for q_ in range(2):
                        nt = g * 2 + q_
                        nc.tensor.matmul(
                            ps[:, q_, :],
                            x_sb[0][:, nt * 128 : (nt + 1) * 128],
                            st["w2ts"][0], start=True, stop=False,
                        )
                        nc.tensor.matmul(
                            ps[:, q_, :],
                            x_sb[1][:, nt * 128 : (nt + 1) * 128],
                            st["w2ts"][1], start=False, stop=True,
                        )
                    nc.vector.tensor_copy(
                        v2t[g // 8][:, (g % 8) * 2 : (g % 8) * 2 + 2, 0:256], ps
                    )

            def emit_s_phase(C_, q8, k8, qc):
                # S^T per k-tile: ONE fp8 DoubleRow matmul (contraction 256
                # packed as [p, 2]); pairs of k-tiles share one 2-bank psum
                # so exp runs as a single [128, 1024] ACT op into the fp8
                # pT pair.
                pTs = []
                for j in range(16):  # k-tile pairs
                    pT = ppool.tile([128, 2, 512], F8, tag=f"p{j}", name=f"pT{j}")
                    ps = pspair.tile([128, 2, 512], F32, tag="mm", name="psS")
                    for i in range(2):
                        kt = 2 * j + i
                        nc.tensor.matmul(
                            ps[:, i, :], k8[:, :, kt * 128 : (kt + 1) * 128],
                            q8[:, :, qc * 512 : (qc + 1) * 512],
                            start=True, stop=True,
                            perf_mode=mybir.MatmulPerfMode.DoubleRow,
                        )
                    nc.scalar.activation(
                        out=pT, in_=ps, func=AF.Exp,
                        scale=1.0 / 16.0, bias=C_["ebias_sb"],
                    )
                    pTs.append(pT)
                return pTs

            def emit_pv_phase(C_, st, v2t, qc, pTs):
                # PV s-outer (one po accumulator at a time, PSUM-friendly):
                # po accumulates [q, 256 c + l] over all 16 pairs via
                # DoubleRow; epilogue (1/l scale) on DVE.
                y_sb = st["y_sb"]
                o_sbs = []
                for s in range(4):
                    po = psov.tile([128, NV2], F32, tag="o", name="po")
                    for j in range(16):
                        nc.tensor.matmul(
                            po,
                            pTs[j][:, :, s * 128 : (s + 1) * 128],
                            v2t[j // 8][:, (j % 8) * 2 : (j % 8) * 2 + 2, :],
                            start=(j == 0),
                            stop=(j == 15),
                            perf_mode=mybir.MatmulPerfMode.DoubleRow,
                            skip_group_check=True,
                        )
                    rl = small.tile([128, 1], F32, tag="rl")
                    nc.vector.reciprocal(rl, po[:, 256:257])
                    o_sb = opool.tile([128, 256], FR, tag="osb")
                    nc.vector.tensor_scalar_mul(o_sb, po[:, 0:256], rl)
                    o_sbs.append(o_sb)
                # transposes DEFERRED past all four po streams: the PE pays
                # the DVE o_sb-latency wait once per qc instead of once per
                # s, so the next S phase (feeding the exp stream) starts
                # ~2 us earlier.
                for s in range(4):
                    for t in range(2):
                        pst = pstp.tile([128, 128], FR, tag="pst", name="pst")
                        nc.tensor.transpose(
                            pst, o_sbs[s][:, t * 128 : (t + 1) * 128], C_["ident"]
                        )
                        ys = y_sb[t][:, qc * 512 + s * 128 : qc * 512 + (s + 1) * 128]
                        nc.vector.tensor_tensor(ys, pst[:, :].bitcast(F32), ys, ALU.add)
                for t in range(2):
                    nc.sync.dma_start(
                        out=y[t * 128 : (t + 1) * 128, qc * 512 : (qc + 1) * 512],
                        in_=y_sb[t][:, qc * 512 : (qc + 1) * 512],
                    )

            # ---------- cross-iteration software pipeline ----------
            # x(N+1) DMAs issue after PV0(N); stats/fold(N+1) is emitted
            # after S3(N) and proj(N+1) after PV2(N), so iteration N+1's
            # prep work executes during N's exp stream instead of queueing
            # behind N's entire engine tails. The ACT exp stream (the
            # bottleneck) then runs back-to-back across iterations.
            C_ = setup()
            v2bufs = []
            for b in range(2):
                pair = []
                for h in range(2):
                    v2b = singles.tile([128, 16, NV2], F8, tag=f"v2b{b}{h}")
                    nc.gpsimd.dma_start(
                        out=v2b[:, :, 256:NV2],
                        in_=ones_in[:, : 16 * NPAD].rearrange("p (f o) -> p f o", o=NPAD),
                    )
                    pair.append(v2b)
                v2bufs.append(pair)
            st = fold_part2(C_, stats_part1(C_, load_x(0)))
            qk = emit_proj(C_, st)
            pTs0 = emit_s_phase(C_, qk[0], qk[1], 0)
            for rep in range(loop_reps):
                q8, k8 = qk
                last = rep + 1 >= loop_reps
                v2t = v2bufs[rep % 2]
                emit_v2(C_, st, v2t)
                x_next = None if last else load_x(rep + 1)
                pTs1 = emit_s_phase(C_, q8, k8, 1)
                emit_pv_phase(C_, st, v2t, 0, pTs0)
                p1 = None if last else stats_part1(C_, x_next)
                pTs2 = emit_s_phase(C_, q8, k8, 2)
                emit_pv_phase(C_, st, v2t, 1, pTs1)
                st_next = None if last else fold_part2(C_, p1)
                pTs3 = emit_s_phase(C_, q8, k8, 3)
                emit_pv_phase(C_, st, v2t, 2, pTs2)
                qk_next = None if last else emit_proj(C_, st_next)
                # next iteration's S(qc0) goes ahead of PV3 so the exp
                # stream crosses the iteration boundary without a PE gap
                pTs0 = (
                    None if last
                    else emit_s_phase(C_, qk_next[0], qk_next[1], 0)
                )
                emit_pv_phase(C_, st, v2t, 3, pTs3)
                st, qk = st_next, qk_next

    return nc


def _get_runner(loop_reps=1):
    key = ("runner", loop_reps)
    if key not in _CACHE:
        nc = _build_nc(loop_reps)
        _CACHE[key] = nc
    return _CACHE[key]


K_USE_FP8 = USE_FP8_PV


def make_extra_inputs():
    gind = np.zeros((128, 16), dtype=np.float32)
    hind = np.zeros((8, 256), dtype=np.float32)
    for t in range(2):
        for p in range(128):
            g = (t * 128 + p) // CPG
            gind[p, t * 8 + g] = 1.0
            hind[g, t * 128 + p] = 1.0
    op = np.zeros((128, 256), dtype=np.float32)
    op[:, 0::16 if USE_FP8_PV else 2] = 1.0
    if USE_FP8_PV:
        import ml_dtypes

        op = op.astype(ml_dtypes.float8_e4m3)
    return {"gind_in": gind, "hind_in": hind, "ones_in": op,
            "ident_in": np.eye(128, dtype=np.float32),
            "onesr_in": np.ones((1, 512), dtype=np.float32)}


def make_weight_inputs(qkv_w, out_w):
    return {
        "wqkT_in": np.ascontiguousarray(qkv_w[0:512].T),
        "owT_in": np.ascontiguousarray(out_w.T),
    }


def kernel(x, gn_gamma, gn_beta, qkv_w, qkv_b, out_w, out_b):
    from concourse.bass_utils import run_bass_kernel_spmd

    x = np.asarray(x, dtype=np.float32)
    gn_gamma = np.asarray(gn_gamma, dtype=np.float32)
    gn_beta = np.asarray(gn_beta, dtype=np.float32)
    qkv_w = np.asarray(qkv_w, dtype=np.float32)
    qkv_b = np.asarray(qkv_b, dtype=np.float32)
    out_w = np.asarray(out_w, dtype=np.float32)
    out_b = np.asarray(out_b, dtype=np.float32)

    b, c, h, w = x.shape
    assert (b, c, h * w) == (B, C, HW)
    xf = x.reshape(b, c, HW)

    nc = _get_runner()
    in_maps = []
    for j in range(N_CORES):
        bi, qh = j // 2, j % 2
        if qh == 0:
            xbj = np.ascontiguousarray(xf[bi])
        else:
            xbj = np.concatenate([xf[bi][:, NQ:], xf[bi][:, :NQ]], axis=1)
        in_maps.append(
            {
                "xb": xbj,
                "qkv_w": qkv_w,
                "qkv_b": qkv_b,
                "out_w": out_w,
                "out_b": out_b,
                "gn_gamma": gn_gamma,
                "gn_beta": gn_beta,
            }
        )
    extras = make_extra_inputs()
    extras.update(make_weight_inputs(qkv_w, out_w))
    for m in in_maps:
        m.update(extras)
    res = run_bass_kernel_spmd(nc, in_maps, core_ids=list(range(N_CORES)))
    out = np.empty((B, C, HW), dtype=np.float32)
    for j in range(N_CORES):
        bi, qh = j // 2, j % 2
        out[bi][:, qh * NQ : (qh + 1) * NQ] = res.results[j]["y"]
    return out.reshape(b, c, h, w)

